# revision 26
# baseline (speedup 1.0000x reference)
"""Trainium2 Bass kernel for 2-layer GATv2 (nn_EvenLamerGAT).

v3 strategy (8 NeuronCores, SPMD single launch):
  - Host: append self-loops, sort edges by dst, partition dst nodes into 8
    contiguous ranges with ~equal edge counts. Each core owns one dst range.
  - Layer-1 gather table (xl1 = x @ W1l, bf16, no bias) is computed locally
    and in full by EVERY core from a replicated, shard-permuted bf16 x^T
    input -- no AllGather and no barrier. bl1 is folded into xr (for the
    attention logits) and into the output bias (softmax weights sum to 1).
  - Layer-1 features use a HEAD-MINOR layout (f = c*8 + h) so per-edge DVE
    broadcasts run in the 2x packed mode (weights/biases permuted host-side).
  - Per dst-block of 128 nodes: dma_gather xl rows per edge (1024 idxs per
    call, 4 SWDGE queues round-robin), load host-precomputed one-hot ST
    tiles from DRAM, build S via one bf16 is_equal per block, and do the
    segment softmax + aggregation with PE matmuls.
  - Logit reduce over channels uses a log2 tree of packed bf16 adds instead
    of tensor_reduce (which has no 2x mode).
  - ELU + the layer-2 node transforms are fused into the phase-B block loop
    (the ELU "-1" is folded into bl2/br2 host-side), so the AllGather input
    is complete when the loop ends; layer 2 then repeats the edge loop.

All schedule shapes are derived from the actual edge_index; the Bass
program is compiled per call.
"""
import os
import sys

sys.path.insert(0, "/opt/trn_rl_repo")

import numpy as np
import ml_dtypes

from concourse import bass, mybir, bacc, tile
from concourse import bass_utils

F32 = mybir.dt.float32
BF16 = mybir.dt.bfloat16
I16 = mybir.dt.int16
AF = mybir.ActivationFunctionType
OP = mybir.AluOpType

NCORES = 8
SPLIT = 32768          # int16 gather index limit
NEG_SLOPE = 0.2
H, C = 8, 32
HC = H * C             # 256
DIN = 128
DOUT = 64
GATHER_TILES_PER_CALL = 8   # 1024 idxs per dma_gather (hard per-call cap)
NQ = 4                 # SWDGE queues

# head-minor permutation: new feature index c*8+h holds old index h*32+c
PERM_HM = np.arange(HC).reshape(H, C).T.reshape(-1)  # perm[new] = old


def _wrap_idx16(idx, num):
    """Wrap `num` int16 indices into the [128, num//16] dma_gather layout."""
    assert num % 128 == 0 and len(idx) == num
    w = np.zeros((128, num // 16), np.int16)
    blk = idx.reshape(num // 16, 16).T
    for g in range(8):
        w[g * 16:(g + 1) * 16, :] = blk
    return w


def _prep_host(x, edge_index):
    N = x.shape[0]
    src = np.concatenate([edge_index[0], np.arange(N, dtype=np.int64)]).astype(np.int64)
    dst = np.concatenate([edge_index[1], np.arange(N, dtype=np.int64)]).astype(np.int64)
    order = np.argsort(dst, kind="stable")
    src_s = src[order].astype(np.int64)
    dst_s = dst[order].astype(np.int64)
    Etot = len(src_s)

    # core ranges: contiguous node spans with ~equal edge counts
    deg = np.bincount(dst_s, minlength=N)
    cum = np.cumsum(deg)
    starts = [0]
    for k in range(1, NCORES):
        starts.append(int(np.searchsorted(cum, k * Etot / NCORES)))
    starts.append(N)
    starts = np.array(starts, np.int64)
    counts = starts[1:] - starts[:-1]
    Np = int(np.ceil(counts.max() / 128) * 128)
    NB = Np // 128
    assert NCORES * Np < 2 * SPLIT, "row index must fit int16 after lo/hi split"

    # global node -> replicated-table row
    owner = np.searchsorted(starts[1:], np.arange(N), side="right")
    table_row = owner * Np + (np.arange(N) - starts[owner])
    src_row = table_row[src_s]
    edge_start = np.searchsorted(dst_s, starts[:-1])
    edge_end = np.searchsorted(dst_s, starts[1:])

    # First pass: per (core, block) lo/hi edge lists (row, dstl)
    per_block = []  # [core][block] = (lo_rows, lo_dstl, hi_rows, hi_dstl)
    T_LO = np.ones(NB, np.int64)
    T_HI = np.zeros(NB, np.int64)
    for c in range(NCORES):
        s0, n_c = starts[c], counts[c]
        blocks = []
        e0, e1 = edge_start[c], edge_end[c]
        er = src_row[e0:e1]
        ed = dst_s[e0:e1] - s0            # local dst 0..n_c-1
        for b in range(NB):
            lo_d, hi_d = b * 128, (b + 1) * 128
            m = (ed >= lo_d) & (ed < hi_d)
            rows = er[m]
            dl = (ed[m] - lo_d).astype(np.float32)
            lo = rows < SPLIT
            lo_rows = rows[lo].astype(np.int64)
            lo_dstl = dl[lo]
            hi_rows = rows[~lo] - SPLIT
            hi_dstl = dl[~lo]
            # dummy edges so padded dst slots have nonzero denominators
            nreal = max(0, min(128, n_c - lo_d))
            if nreal < 128:
                pad_d = np.arange(nreal, 128, dtype=np.float32)
                lo_rows = np.concatenate([lo_rows, np.zeros(len(pad_d), np.int64)])
                lo_dstl = np.concatenate([lo_dstl, pad_d])
            blocks.append((lo_rows, lo_dstl, hi_rows, hi_dstl))
            T_LO[b] = max(T_LO[b], (len(lo_rows) + 127) // 128)
            T_HI[b] = max(T_HI[b], (len(hi_rows) + 127) // 128)
        per_block.append(blocks)

    TB = T_LO + T_HI                       # per-block tile count
    dcol = np.zeros(NB + 1, np.int64)      # dstl column offset per block
    dcol[1:] = np.cumsum(TB)
    TCOLS = int(dcol[-1])
    # Second pass: fill padded arrays
    gidx = np.zeros((NCORES, 128, TCOLS * 8), np.int16)
    dstl = np.full((NCORES, 128, TCOLS), 300.0, np.float32)
    for c in range(NCORES):
        for b in range(NB):
            lo_rows, lo_dstl, hi_rows, hi_dstl = per_block[c][b]
            for (rows, dls, toff, nt) in (
                (lo_rows, lo_dstl, 0, int(T_LO[b])),
                (hi_rows, hi_dstl, int(T_LO[b]), int(T_HI[b])),
            ):
                n = nt * 128
                ridx = np.zeros(n, np.int64)
                ridx[: len(rows)] = rows
                dpad = np.full(n, 300.0, np.float32)
                dpad[: len(dls)] = dls
                c0 = int(dcol[b]) + toff
                # dstl layout: slot k=(t*128+p) -> [p, c0+t]
                dstl[c][:, c0:c0 + nt] = dpad.reshape(nt, 128).T
                # gather idx layout: wrapped per call chunk
                for t0 in range(0, nt, GATHER_TILES_PER_CALL):
                    ntc = min(GATHER_TILES_PER_CALL, nt - t0)
                    chunk = ridx[t0 * 128:(t0 + ntc) * 128].astype(np.int16)
                    col0 = (c0 + t0) * 8
                    gidx[c][:, col0:col0 + ntc * 8] = _wrap_idx16(chunk, ntc * 128)

    # one-hot ST tiles [d, e] per (core, tile-col): STb[c][:, t*128+e] = 1 iff
    # dstl[c][e, t] == d.  Sentinel 300 -> all-zero column.
    stb = np.zeros((NCORES, 128, TCOLS * 128), ml_dtypes.bfloat16)
    for c in range(NCORES):
        dl = dstl[c]                       # [128(e-slot), TCOLS]
        for t in range(TCOLS):
            e_sel = np.nonzero(dl[:, t] < 128)[0]
            d_sel = dl[e_sel, t].astype(np.int64)
            col = t * 128 + e_sel
            stb[c][d_sel, col] = 1.0

    # shard-permuted x^T (replicated table order), bf16
    xT = np.ascontiguousarray(x.T)  # [128, N]
    NROWS = NCORES * Np
    xT_perm = np.zeros((DIN, NROWS), ml_dtypes.bfloat16)
    for c in range(NCORES):
        xT_perm[:, c * Np:c * Np + counts[c]] = xT[:, starts[c]:starts[c] + counts[c]]
    xrT_sh = np.zeros((NCORES, DIN, Np), ml_dtypes.bfloat16)
    for c in range(NCORES):
        xrT_sh[c] = xT_perm[:, c * Np:(c + 1) * Np]

    return dict(N=N, starts=starts, counts=counts, Np=Np, NB=NB,
                T_LO=T_LO, T_HI=T_HI, TB=TB, dcol=dcol, TCOLS=TCOLS,
                gidx=gidx, dstl=dstl.astype(ml_dtypes.bfloat16), stb=stb,
                xT_perm=xT_perm, xrT_sh=xrT_sh)


def _build_bass(P):
    """Build the SPMD Bass program for prep dict P."""
    Np, NB = P["Np"], P["NB"]
    T_LO, T_HI, TB, dcol, TCOLS = P["T_LO"], P["T_HI"], P["TB"], P["dcol"], P["TCOLS"]
    TMAX = int(TB.max())
    NROWS = NCORES * Np
    NT = NROWS // 128

    nc = bacc.Bacc("TRN2", target_bir_lowering=False, debug=False,
                   enable_asserts=True, num_devices=NCORES, num_swdge_queues=NQ)

    din = lambda n, s, d: nc.dram_tensor(n, s, d, kind="ExternalInput").ap()
    xT_in = din("xT", [DIN, NROWS], BF16)
    xrT_in = din("xrT", [DIN, Np], BF16)
    st_in = din("stb", [128, TCOLS * 128], BF16)
    gidx_in = din("gidx", [128, TCOLS * 8], I16)
    dstl_in = din("dstl", [128, TCOLS], BF16)
    iotab_in = din("iotab", [128, 128], BF16)
    identb_in = din("identb", [128, 128], BF16)
    att1_in = din("att1r", [128, HC], F32)
    att2_in = din("att2r", [128, DOUT], F32)
    w1l_in = din("w1l", [DIN, HC], F32)
    w1r_in = din("w1r", [DIN, HC], F32)
    w2l_in = din("w2l", [HC, DOUT], F32)
    w2r_in = din("w2r", [HC, DOUT], F32)
    br1_in = din("br1r", [128, HC], F32)    # br1 + bl1
    b1_in = din("b1r", [128, HC], F32)      # bias1 + bl1
    bl2_in = din("bl2r", [128, DOUT], F32)
    br2_in = din("br2r", [128, DOUT], F32)
    b2_in = din("b2r", [128, DOUT], F32)
    out1 = nc.dram_tensor("out1", [Np, DOUT], F32, kind="ExternalOutput").ap()
    out2 = nc.dram_tensor("out2", [Np, DOUT], F32, kind="ExternalOutput").ap()

    qctr = [0]

    def next_q():
        q = qctr[0] % NQ
        qctr[0] += 1
        return q

    PREP = os.environ.get("KERNEL_PREP", "0") == "1"

    with tile.TileContext(nc) as tc:
        qsems = [nc.alloc_semaphore(f"gq{q}") for q in range(NQ)] if PREP else None

        def gather(out_ap, src_ap, idx_cols_ap, nidx):
            q = next_q()
            if PREP:
                nc.gpsimd.dma_gather(
                    out_ap=out_ap, in_ap=src_ap, idxs_ap=idx_cols_ap,
                    num_idxs=nidx, num_idxs_reg=nidx,
                    elem_size=out_ap.shape[-1], queue_num=q,
                    prepare_only=True, sem=qsems[q],
                )
                nc.gpsimd.trigger_dma(count=None, queue_num=q)
            else:
                nc.gpsimd.dma_gather(
                    out_ap=out_ap, in_ap=src_ap, idxs_ap=idx_cols_ap,
                    num_idxs=nidx, num_idxs_reg=nidx,
                    elem_size=out_ap.shape[-1], queue_num=q,
                )

        import contextlib
        with contextlib.ExitStack() as ctx:
            cn = ctx.enter_context(tc.tile_pool(name="const", bufs=1))
            dr = ctx.enter_context(tc.tile_pool(name="dram", bufs=1, space="DRAM"))

            def load_const(ap_in, shape, dt, cast=False):
                t = cn.tile(shape, dt, tag=ap_in.tensor.name)
                (nc.gpsimd if cast else nc.sync).dma_start(out=t[:], in_=ap_in[:])
                return t

            iotab = load_const(iotab_in, [128, 128], BF16)
            identb = load_const(identb_in, [128, 128], BF16)
            att1b = load_const(att1_in, [128, HC], BF16, cast=True)
            att2f = load_const(att2_in, [128, DOUT], BF16, cast=True)
            w1l = load_const(w1l_in, [DIN, HC], BF16, cast=True)
            w1r = load_const(w1r_in, [DIN, HC], BF16, cast=True)
            br1 = load_const(br1_in, [128, HC], F32)
            b1r = load_const(b1_in, [128, HC], F32)
            bl2 = load_const(bl2_in, [128, DOUT], F32)
            br2 = load_const(br2_in, [128, DOUT], F32)
            b2r = load_const(b2_in, [128, DOUT], F32)
            # W2 as [128, 2, DOUT] bf16 (rows 0:128, 128:256)
            w2lb = cn.tile([128, 2, DOUT], BF16)
            nc.gpsimd.dma_start(out=w2lb[:, 0, :], in_=w2l_in[0:128, :])
            nc.gpsimd.dma_start(out=w2lb[:, 1, :], in_=w2l_in[128:256, :])
            w2rb = cn.tile([128, 2, DOUT], BF16)
            nc.gpsimd.dma_start(out=w2rb[:, 0, :], in_=w2r_in[0:128, :])
            nc.gpsimd.dma_start(out=w2rb[:, 1, :], in_=w2r_in[128:256, :])
            gidx_sb = cn.tile([128, TCOLS * 8], I16)
            nc.sync.dma_start(out=gidx_sb[:], in_=gidx_in[:])
            dstl_sb = cn.tile([128, TCOLS], BF16)
            nc.sync.dma_start(out=dstl_sb[:], in_=dstl_in[:])

            # DRAM buffers
            xl_full = dr.tile([NROWS, HC], BF16)
            xl2_ag_in = dr.tile([Np, 2 * DOUT], BF16)
            xl2_full = dr.tile([NROWS, 2 * DOUT], BF16, addr_space="Shared")

            persist1 = ctx.enter_context(tc.tile_pool(name="persist1", bufs=1))
            xr_sh = persist1.tile([128, NB, HC], BF16)
            h_sh = persist1.tile([128, NB, HC], BF16)
            persist2 = ctx.enter_context(tc.tile_pool(name="persist2", bufs=1))
            xr2_sh = persist2.tile([128, NB, DOUT], BF16)
            o1_sh = persist2.tile([128, NB, DOUT], F32)

            # ---- Phase A: full xl1 table (all rows), own-shard xr ----
            GA = 4
            with (
                nc.named_scope("phaseA"),
                tc.tile_pool(name="pa_sb", bufs=3) as pa,
                tc.tile_pool(name="pa_ps", bufs=2, space="PSUM") as pap,
            ):
                cpcnt = [0]
                for i0 in range(0, NT, GA):
                    ng = min(GA, NT - i0)
                    xt4 = pa.tile([128, GA, 128], BF16, tag="xt4")
                    nc.sync.dma_start(
                        out=xt4[:, :ng, :],
                        in_=xT_in[:, i0 * 128:(i0 + ng) * 128].rearrange(
                            "p (g q) -> p g q", g=ng))
                    psa = pap.tile([128, GA, HC], F32, space="PSUM", tag="psa")
                    for j in range(ng):
                        nc.tensor.matmul(out=psa[:, j, :], lhsT=xt4[:, j, :],
                                         rhs=w1l[:], start=True, stop=True)
                    xlt4 = pa.tile([128, GA, HC], BF16, tag="xlt4")
                    eng = nc.scalar.copy if cpcnt[0] % 2 else nc.vector.tensor_copy
                    cpcnt[0] += 1
                    eng(out=xlt4[:, :ng, :], in_=psa[:, :ng, :])
                    nc.sync.dma_start(
                        out=xl_full[i0 * 128:(i0 + ng) * 128, :].rearrange(
                            "(g p) h -> p g h", p=128),
                        in_=xlt4[:, :ng, :])
                # own-shard xr (+ br1 + bl1)
                for i in range(NB):
                    xt = pa.tile([128, 128], BF16, tag="xt")
                    nc.sync.dma_start(out=xt[:], in_=xrT_in[:, i * 128:(i + 1) * 128])
                    psr = pap.tile([128, HC], F32, space="PSUM", tag="psr")
                    nc.tensor.matmul(out=psr[:], lhsT=xt[:], rhs=w1r[:], start=True, stop=True)
                    nc.vector.tensor_tensor(out=xr_sh[:, i, :], in0=psr[:], in1=br1[:], op=OP.add)

            # ---- Phase B: layer-1 edge loop (+ fused ELU / layer-2 node
            # transforms per block, so the AllGather input is ready at the
            # end of the loop and no serial phase-C gap remains) ----
            with (
                nc.named_scope("phaseB"),
                tc.tile_pool(name="pb_sb", bufs=2) as pb,
                tc.tile_pool(name="pb_g", bufs=3) as pbg,
                tc.tile_pool(name="pb_st", bufs=3) as pbst,
                tc.tile_pool(name="pb_s", bufs=2) as pbs,
                tc.tile_pool(name="pb_mp", bufs=2, space="PSUM") as pbmp,
                tc.tile_pool(name="pb_agg", bufs=1, space="PSUM") as pbagg,
                tc.tile_pool(name="pb_c", bufs=2) as pbc,
                tc.tile_pool(name="pb_cp", bufs=1, space="PSUM") as pbcp,
            ):
                for b in range(NB):
                    tlo, thi, tb = int(T_LO[b]), int(T_HI[b]), int(TB[b])
                    c0 = int(dcol[b])
                    sts = pbst.tile([128, TMAX, 128], BF16, tag="sts")
                    nc.sync.dma_start(
                        out=sts[:, :tb, :],
                        in_=st_in[:, c0 * 128:(c0 + tb) * 128].rearrange(
                            "p (t q) -> p t q", t=tb))
                    xlg = pbg.tile([128, TMAX, HC], BF16, tag="xlg")
                    for (toff, nt, base) in ((0, tlo, 0), (tlo, thi, SPLIT)):
                        if nt == 0:
                            continue
                        src_ap = xl_full[:] if base == 0 else xl_full[base:, :]
                        for t0 in range(0, nt, GATHER_TILES_PER_CALL):
                            ntc = min(GATHER_TILES_PER_CALL, nt - t0)
                            col0 = (c0 + toff + t0) * 8
                            gather(xlg[:, toff + t0:toff + t0 + ntc, :], src_ap,
                                   gidx_sb[:, col0:col0 + ntc * 8], ntc * 128)
                    S_blk = pbs.tile([128, TMAX, 128], BF16, tag="Sb")
                    nc.vector.tensor_tensor(
                        out=S_blk[:, :tb, :],
                        in0=dstl_sb[:, c0:c0 + tb, None].to_broadcast([128, tb, 128]),
                        in1=iotab[:, None, :].to_broadcast([128, tb, 128]),
                        op=OP.is_equal)
                    e_blk = pb.tile([128, TMAX, H], F32, tag="e")
                    am_blk = pb.tile([128, TMAX, HC], BF16, tag="am")
                    # pass 1: m (grouped identity mm), prelu, lr*att
                    for g0 in range(0, tb, 4):
                        gs = min(4, tb - g0)
                        lr4 = pb.tile([128, 4, HC], BF16, tag="lr4")
                        for j0 in range(0, gs, 2):
                            js = min(2, gs - j0)
                            mp = pbmp.tile([128, 2, 512], F32, space="PSUM", tag="mp")
                            for j in range(js):
                                nc.tensor.matmul(out=mp[:, j, 0:HC], lhsT=sts[:, g0 + j0 + j, :],
                                                 rhs=xr_sh[:, b, :], start=True, stop=False)
                            for j in range(js):
                                nc.tensor.matmul(out=mp[:, j, 0:HC], lhsT=identb[:],
                                                 rhs=xlg[:, g0 + j0 + j, :], start=False, stop=True)
                            nc.scalar.activation(
                                out=lr4[:, j0:j0 + js, :],
                                in_=mp[:, :js, 0:HC], func=AF.Prelu, alpha=NEG_SLOPE)
                        nc.vector.tensor_tensor(
                            out=am_blk[:, g0:g0 + gs, :], in0=lr4[:, :gs, :],
                            in1=att1b[:, None, :].to_broadcast([128, gs, HC]), op=OP.mult)
                    # in-place tree-reduce head-minor: am [p, t, 32, 8] -> e [p, t, 8]
                    amv = am_blk[:, :tb, :].rearrange("p t (c h) -> p t c h", h=H)
                    nc.vector.tensor_tensor(
                        out=amv[:, :, 0:16, :], in0=amv[:, :, 0:16, :],
                        in1=amv[:, :, 16:32, :], op=OP.add)
                    nc.vector.tensor_tensor(
                        out=amv[:, :, 0:8, :], in0=amv[:, :, 0:8, :],
                        in1=amv[:, :, 8:16, :], op=OP.add)
                    nc.vector.tensor_tensor(
                        out=amv[:, :, 0:4, :], in0=amv[:, :, 0:4, :],
                        in1=amv[:, :, 4:8, :], op=OP.add)
                    nc.vector.tensor_tensor(
                        out=amv[:, :, 0:2, :], in0=amv[:, :, 0:2, :],
                        in1=amv[:, :, 2:4, :], op=OP.add)
                    nc.vector.tensor_tensor(
                        out=e_blk[:, :tb, :], in0=amv[:, :, 0, :],
                        in1=amv[:, :, 1, :], op=OP.add)
                    exb = pb.tile([128, TMAX, H], BF16, tag="ex")
                    nc.scalar.activation(
                        out=exb[:, :tb, :].rearrange("p t h -> p (t h)"),
                        in_=e_blk[:, :tb, :].rearrange("p t h -> p (t h)"), func=AF.Exp)
                    # pass 2: xlx = xl * ex (head-minor broadcast), agg, den
                    xlx = pb.tile([128, TMAX, HC + H], BF16, tag="xlx")
                    nc.vector.tensor_tensor(
                        out=xlx[:, :tb, 0:HC].rearrange("p t (c h) -> p t c h", h=H),
                        in0=xlg[:, :tb, :].rearrange("p t (c h) -> p t c h", h=H),
                        in1=exb[:, :tb, None, :].to_broadcast([128, tb, C, H]),
                        op=OP.mult)
                    nc.scalar.copy(out=xlx[:, :tb, HC:HC + H], in_=exb[:, :tb, :])
                    agg = pbagg.tile([128, HC + H], F32, space="PSUM", tag="agg")
                    for t in range(tb):
                        nc.tensor.matmul(out=agg[:], lhsT=S_blk[:, t, :],
                                         rhs=xlx[:, t, :],
                                         start=(t == 0), stop=(t == tb - 1))
                    rd = pb.tile([128, H], F32, tag="rd")
                    nc.vector.reciprocal(out=rd[:], in_=agg[:, HC:HC + H])
                    nc.vector.tensor_tensor(
                        out=h_sh[:, b, :].rearrange("p (c h) -> p c h", h=H),
                        in0=agg[:, 0:HC].rearrange("p (c h) -> p c h", h=H),
                        in1=rd[:, None, :].to_broadcast([128, C, H]), op=OP.mult)

                    # fused bias1 + ELU for this block
                    view = h_sh[:, b, :]
                    nc.vector.tensor_tensor(out=view, in0=view, in1=b1r[:], op=OP.add)
                    negt = pbc.tile([128, HC], F32, tag="neg")
                    nc.vector.tensor_scalar(out=negt[:], in0=view, scalar1=0.0,
                                            scalar2=None, op0=OP.min)
                    expt = pbc.tile([128, HC], F32, tag="exp")
                    nc.scalar.activation(out=expt[:], in_=negt[:], func=AF.Exp)
                    # h := max(x,0) + exp(min(x,0));  the ELU "-1" is folded
                    # into bl2/br2 host-side (subtracting W2 column sums)
                    nc.vector.tensor_scalar(out=view, in0=view, scalar1=0.0,
                                            scalar2=None, op0=OP.max)
                    nc.vector.tensor_tensor(out=view, in0=view, in1=expt[:], op=OP.add)

                    # fused layer-2 node transforms for this block
                    hT = pbc.tile([128, 2, 128], BF16, tag="hT")
                    for half in range(2):
                        tp = pbcp.tile([128, 128], BF16, space="PSUM", tag="tp")
                        nc.tensor.transpose(
                            out=tp[:], in_=h_sh[:, b, half * 128:(half + 1) * 128],
                            identity=identb[:])
                        nc.scalar.copy(out=hT[:, half, :], in_=tp[:])
                    ps2l = pbcp.tile([128, DOUT], F32, space="PSUM", tag="ps2l")
                    ps2r = pbcp.tile([128, DOUT], F32, space="PSUM", tag="ps2r")
                    for half in range(2):
                        nc.tensor.matmul(out=ps2l[:], lhsT=hT[:, half, :], rhs=w2lb[:, half, :],
                                         start=(half == 0), stop=(half == 1))
                        nc.tensor.matmul(out=ps2r[:], lhsT=hT[:, half, :], rhs=w2rb[:, half, :],
                                         start=(half == 0), stop=(half == 1))
                    xl2t = pbc.tile([128, DOUT], BF16, tag="xl2t")
                    nc.vector.tensor_tensor(out=xl2t[:], in0=ps2l[:], in1=bl2[:], op=OP.add)
                    nc.sync.dma_start(out=xl2_ag_in[b * 128:(b + 1) * 128, 0:DOUT], in_=xl2t[:])
                    nc.sync.dma_start(out=xl2_ag_in[b * 128:(b + 1) * 128, DOUT:2 * DOUT], in_=xl2t[:])
                    nc.vector.tensor_tensor(out=xr2_sh[:, b, :], in0=ps2r[:], in1=br2[:], op=OP.add)

            with nc.named_scope("ag2"):
                nc.gpsimd.collective_compute(
                    "AllGather", OP.bypass, replica_groups=[list(range(NCORES))],
                    ins=[xl2_ag_in[:].opt()], outs=[xl2_full[:].opt()],
                )

            # ---- Phase D: layer-2 edge loop (1 head, 64 ch, grouped) ----
            with (
                nc.named_scope("phaseD"),
                tc.tile_pool(name="pd_sb", bufs=3) as pd,
                tc.tile_pool(name="pd_g", bufs=4) as pdg,
                tc.tile_pool(name="pd_st", bufs=3) as pdst,
                tc.tile_pool(name="pd_s", bufs=2) as pds,
                tc.tile_pool(name="pd_mp", bufs=3, space="PSUM") as pdmp,
                tc.tile_pool(name="pd_agg", bufs=2, space="PSUM") as pdagg,
            ):
                for b in range(NB):
                    tlo, thi, tb = int(T_LO[b]), int(T_HI[b]), int(TB[b])
                    c0 = int(dcol[b])
                    sts = pdst.tile([128, TMAX, 128], BF16, tag="sts2")
                    nc.sync.dma_start(
                        out=sts[:, :tb, :],
                        in_=st_in[:, c0 * 128:(c0 + tb) * 128].rearrange(
                            "p (t q) -> p t q", t=tb))
                    x2g = pdg.tile([128, TMAX, 2 * DOUT], BF16, tag="x2g")
                    for (toff, nt, base) in ((0, tlo, 0), (tlo, thi, SPLIT)):
                        if nt == 0:
                            continue
                        src_ap = xl2_full[:] if base == 0 else xl2_full[base:, :]
                        for t0 in range(0, nt, GATHER_TILES_PER_CALL):
                            ntc = min(GATHER_TILES_PER_CALL, nt - t0)
                            col0 = (c0 + toff + t0) * 8
                            gather(x2g[:, toff + t0:toff + t0 + ntc, :], src_ap,
                                   gidx_sb[:, col0:col0 + ntc * 8], ntc * 128)
                    S_blk = pds.tile([128, TMAX, 128], BF16, tag="S2b")
                    nc.vector.tensor_tensor(
                        out=S_blk[:, :tb, :],
                        in0=dstl_sb[:, c0:c0 + tb, None].to_broadcast([128, tb, 128]),
                        in1=iotab[:, None, :].to_broadcast([128, tb, 128]),
                        op=OP.is_equal)
                    e_blk = pd.tile([128, TMAX], F32, tag="e2")
                    lr_blk = pd.tile([128, TMAX, DOUT], BF16, tag="lr2")
                    for g0 in range(0, tb, 4):
                        gs = min(4, tb - g0)
                        for j0 in range(0, gs, 2):
                            js = min(2, gs - j0)
                            mp = pdmp.tile([128, 2, 512], F32, space="PSUM", tag="mp2")
                            for j in range(js):
                                nc.tensor.matmul(out=mp[:, j, 0:DOUT], lhsT=sts[:, g0 + j0 + j, :],
                                                 rhs=xr2_sh[:, b, :], start=True, stop=False)
                            for j in range(js):
                                nc.tensor.matmul(out=mp[:, j, 0:DOUT], lhsT=identb[:],
                                                 rhs=x2g[:, g0 + j0 + j, 0:DOUT], start=False, stop=True)
                            nc.scalar.activation(
                                out=lr_blk[:, g0 + j0:g0 + j0 + js, :],
                                in_=mp[:, :js, 0:DOUT], func=AF.Prelu, alpha=NEG_SLOPE)
                        am4 = pd.tile([128, 4, DOUT], BF16, tag="am24")
                        nc.vector.tensor_tensor(
                            out=am4[:, :gs, :], in0=lr_blk[:, g0:g0 + gs, :],
                            in1=att2f[:, None, :].to_broadcast([128, gs, DOUT]), op=OP.mult)
                        nc.vector.tensor_reduce(
                            out=e_blk[:, g0:g0 + gs], in_=am4[:, :gs, :],
                            axis=mybir.AxisListType.X, op=OP.add)
                    exb = pd.tile([128, TMAX], BF16, tag="ex2")
                    nc.scalar.activation(out=exb[:, :tb], in_=e_blk[:, :tb], func=AF.Exp)
                    agg = pdagg.tile([128, DOUT + 1], F32, space="PSUM", tag="agg2")
                    xlx2 = pd.tile([128, TMAX, DOUT + 1], BF16, tag="xlx2")
                    nc.vector.tensor_tensor(
                        out=xlx2[:, :tb, 0:DOUT], in0=x2g[:, :tb, 0:DOUT],
                        in1=exb[:, :tb, None].to_broadcast([128, tb, DOUT]),
                        op=OP.mult)
                    nc.scalar.copy(out=xlx2[:, :tb, DOUT:DOUT + 1],
                                   in_=exb[:, :tb, None])
                    for t in range(tb):
                        nc.tensor.matmul(out=agg[:], lhsT=S_blk[:, t, :], rhs=xlx2[:, t, :],
                                         start=(t == 0), stop=(t == tb - 1))
                    rd = pd.tile([128, 1], F32, tag="rd2")
                    nc.vector.reciprocal(out=rd[:], in_=agg[:, DOUT:DOUT + 1])
                    nc.vector.tensor_tensor(
                        out=o1_sh[:, b, :], in0=agg[:, 0:DOUT],
                        in1=rd[:, 0:1].to_broadcast([128, DOUT]), op=OP.mult)

            # ---- Phase E: bias2 + outputs + log_softmax ----
            with nc.named_scope("phaseE"), tc.tile_pool(name="pf_sb", bufs=1) as pf:
                nc.vector.tensor_tensor(
                    out=o1_sh[:], in0=o1_sh[:],
                    in1=b2r[:, None, :].to_broadcast([128, NB, DOUT]), op=OP.add)
                nc.sync.dma_start(
                    out=out1.rearrange("(b p) c -> p b c", p=128), in_=o1_sh[:])
                rmax = pf.tile([128, NB], F32)
                nc.vector.tensor_reduce(out=rmax[:], in_=o1_sh[:],
                                        axis=mybir.AxisListType.X, op=OP.max)
                xm = pf.tile([128, NB, DOUT], F32)
                nc.vector.tensor_tensor(
                    out=xm[:], in0=o1_sh[:],
                    in1=rmax[:, :, None].to_broadcast([128, NB, DOUT]), op=OP.subtract)
                pexp = pf.tile([128, NB, DOUT], F32)
                nc.scalar.activation(out=pexp[:].rearrange("p b c -> p (b c)"),
                                     in_=xm[:].rearrange("p b c -> p (b c)"), func=AF.Exp)
                ssum = pf.tile([128, NB], F32)
                nc.vector.tensor_reduce(out=ssum[:], in_=pexp[:],
                                        axis=mybir.AxisListType.X, op=OP.add)
                lns = pf.tile([128, NB], F32)
                nc.scalar.activation(out=lns[:], in_=ssum[:], func=AF.Ln)
                nc.vector.tensor_tensor(
                    out=xm[:], in0=xm[:],
                    in1=lns[:, :, None].to_broadcast([128, NB, DOUT]), op=OP.subtract)
                nc.sync.dma_start(
                    out=out2.rearrange("(b p) c -> p b c", p=128), in_=xm[:])

    nc.compile()
    return nc


def _dump_engine_stats(insts, path):
    """Per-engine busy time + top ops."""
    from collections import defaultdict
    eng_busy = defaultdict(int)
    eng_cnt = defaultdict(int)
    lines = []
    for i in insts:
        try:
            d = int(i.duration or 0)
            eng = str(i.engine)
        except Exception:
            continue
        eng_busy[eng] += d
        eng_cnt[eng] += 1
    lines.append("== engine busy (sum dur) ==")
    for eng, b in sorted(eng_busy.items(), key=lambda kv: -kv[1]):
        lines.append(f"  {eng:14s} {b:>10d} ns  n={eng_cnt[eng]}")
    with open(path, "w") as f:
        f.write("\n".join(lines) + "\n")
    print("\n".join(lines))


def kernel(x, edge_index, Wl1, bl1, Wr1, br1, att1, bias1,
           Wl2, bl2, Wr2, br2, att2, bias2):
    x = np.asarray(x, np.float32)
    edge_index = np.asarray(edge_index)
    P = _prep_host(x, edge_index)
    nc = _build_bass(P)

    rep = lambda v, w: np.tile(np.asarray(v, np.float32).reshape(1, -1), (128, 1))[:, :w]
    br1_eff = (np.asarray(br1, np.float32) + np.asarray(bl1, np.float32))[PERM_HM]
    b1_eff = (np.asarray(bias1, np.float32) + np.asarray(bl1, np.float32))[PERM_HM]
    att1_hm = np.asarray(att1, np.float32).reshape(HC)[PERM_HM]
    consts = {
        "iotab": np.tile(np.arange(128, dtype=np.float32)[None, :], (128, 1)).astype(ml_dtypes.bfloat16),
        "identb": np.eye(128).astype(ml_dtypes.bfloat16),
        "att1r": np.tile(att1_hm.reshape(1, HC), (128, 1)),
        "att2r": np.tile(np.asarray(att2, np.float32).reshape(1, DOUT), (128, 1)),
        "w1l": np.asarray(Wl1, np.float32)[:, PERM_HM],
        "w1r": np.asarray(Wr1, np.float32)[:, PERM_HM],
        "w2l": np.asarray(Wl2, np.float32)[PERM_HM, :],
        "w2r": np.asarray(Wr2, np.float32)[PERM_HM, :],
        "br1r": rep(br1_eff, HC), "b1r": rep(b1_eff, HC),
        "bl2r": rep(np.asarray(bl2, np.float32) - np.asarray(Wl2, np.float32).sum(0), DOUT),
        "br2r": rep(np.asarray(br2, np.float32) - np.asarray(Wr2, np.float32).sum(0), DOUT),
        "b2r": rep(bias2, DOUT),
        "xT": P["xT_perm"],
    }
    in_maps = []
    for c in range(NCORES):
        m = dict(consts)
        m["xrT"] = P["xrT_sh"][c]
        m["stb"] = P["stb"][c]
        m["gidx"] = P["gidx"][c]
        m["dstl"] = P["dstl"][c]
        in_maps.append(m)

    trace = bool(os.environ.get("KERNEL_TRACE"))
    res = bass_utils.run_bass_kernel_spmd(
        nc, in_maps, core_ids=list(range(NCORES)), trace=trace)
    kernel.last_res = res
    if trace and res.exec_time_ns:
        print(f"HW exec time: {res.exec_time_ns} ns")
        kernel.last_results = res
        if res.instructions_and_trace:
            insts, tpath = res.instructions_and_trace
            print(f"trace path: {tpath}")
            try:
                _dump_engine_stats(insts, "/tmp/kern_engine_stats.txt")
            except Exception as e:
                print(f"engine stats failed: {e}")

    counts = P["counts"]
    h = np.concatenate([res.results[c]["out1"][:counts[c]] for c in range(NCORES)], axis=0)
    ls = np.concatenate([res.results[c]["out2"][:counts[c]] for c in range(NCORES)], axis=0)
    return h, ls
